# revision 21
# baseline (speedup 1.0000x reference)
"""DLRM pairwise-interaction kernel for Trainium2 (Bass/Tile), 8-core data parallel.

Problem: inputs [B=8192, N=64, D=128] fp32 ->
         out [B, 2016] fp32 = strictly-lower-tri (i-major) of per-sample Gram
         Z_b = X_b @ X_b^T.

Sharding: pure data parallel, B split into 8 shards of 1024 samples.

Per-core plan (1024 samples = 2 obatches of 512 = 8 blocks of 64 samples):
  - DMA in a 64-sample block as [128p, 4096]: partition = (pair-half, n),
    free = (pair c, d). Chunk c is the 2-sample stack [X_{2c}; X_{2c+1}].
  - TensorE transpose (fp32r) each chunk -> PSUM [128(d), 128(s,n)] = [Xa^T|Xb^T].
  - DVE copies 4 transposes at a time PSUM->SBUF (XT tile [128, 512]).
  - Gram matmul per pair (fp32r): lhsT = XT_c [128,128],
    rhs = 2-pair group [128,256] (N=256 hits the fast fp32r path; half the
    output is cross-sample garbage). Output slice is shifted by -(k%2)*128 so
    useful diag blocks land at k*256 + [0:64] (A) / +[64:128] (B) in PSUM.
  - ScalarE extracts diag blocks into Zbig [128, 256*64] for the obatch:
    Zbig[64*a + i, qq*64 + j] = Z_{2qq+a}[i, j], qq = pair index in obatch.
  - 63 out-DMAs per obatch: row index i moves [2 x 256 x i] strided
    (partition {i, 64+i}; sample stride uniform because consecutive pairs are
    adjacent samples) to out[s, T(i):T(i)+i], T(i)=i(i-1)/2.
"""

import numpy as np

import concourse.bass as bass
from concourse import bacc, tile, mybir
from concourse import bass_utils

F32 = mybir.dt.float32
F32R = mybir.dt.float32r

B_FULL = 8192
N_CORES = 8
B_CORE = B_FULL // N_CORES  # 1024
N = 64
D = 128
OUT_COLS = (N * (N - 1)) // 2  # 2016

BLK = 64                       # samples per input block
BLK_PAIRS = BLK // 2           # 32


def _tri(i: int) -> int:
    return (i * (i - 1)) // 2


def build_nc(b_core: int = B_CORE, repeats: int = 1, mode: str = "f32r",
             skip_out: bool = False, ob: int = 512, dma_cast: bool = True):
    """Build the Bass program for one core processing b_core samples.

    repeats > 1 wraps the whole workload in a hardware loop (timing only).
    mode: "f32r" (fp32r matmuls, ~1.6e-4 rel err on HW) or "bf16" (~2.5e-3).
    skip_out: replace packed-tril out-DMAs with one bulk dump (timing only).
    ob: samples per output batch (zbig size; 1024 -> 126 out-DMAs total).
    dma_cast: cast fp32->mm_dt inline in the input SWDGE DMA.
    """
    OB = ob
    OB_PAIRS = OB // 2
    BLKS_PER_OB = OB // BLK
    assert b_core % OB == 0
    n_ob = b_core // OB
    nc = bacc.Bacc("TRN2", target_bir_lowering=False, debug=False,
                   num_devices=N_CORES)
    x = nc.dram_tensor("x", [b_core, N, D], F32, kind="ExternalInput").ap()
    ident = nc.dram_tensor("ident", [128, 128], F32, kind="ExternalInput").ap()
    out = nc.dram_tensor("out", [b_core, OUT_COLS], F32,
                         kind="ExternalOutput").ap()

    bf16 = mode == "bf16"
    mm_dt = mybir.dt.bfloat16 if bf16 else F32R

    with tile.TileContext(nc) as tc:
        with (
            tc.tile_pool(name="xin", bufs=2) as xin_pool,
            tc.tile_pool(name="xbf", bufs=2) as xbf_pool,
            tc.tile_pool(name="xt", bufs=4) as xt_pool,
            tc.tile_pool(name="zbig", bufs=2 if ob <= 512 else 1) as zbig_pool,
            tc.tile_pool(name="const", bufs=1) as const_pool,
            tc.tile_pool(name="pst", bufs=2, space=bass.MemorySpace.PSUM) as pst_pool,
            tc.tile_pool(name="psz", bufs=2, space=bass.MemorySpace.PSUM) as psz_pool,
        ):
            ident_sb = const_pool.tile([128, 128], F32)
            nc.sync.dma_start(ident_sb[:], ident[:])
            # fp32r matmul operands must be explicitly rounded to fp32r
            # (BIR verifier enforces it), so both modes cast via tensor_copy.
            ident_mm = const_pool.tile([128, 128], mm_dt)
            nc.vector.tensor_copy(ident_mm[:], ident_sb[:])

            def body(_iv=None):
                for obi in range(n_ob):
                    zbig = zbig_pool.tile([128, OB_PAIRS * N], F32)
                    for blk in range(BLKS_PER_OB):
                        s0 = obi * OB + blk * BLK
                        src = x[s0:s0 + BLK]
                        src = src.rearrange("(c two) n d -> (two n) c d", two=2)
                        if dma_cast:
                            # SWDGE casts fp32->mm_dt inline during the load
                            xsrc = xbf_pool.tile([128, BLK_PAIRS * D], mm_dt)
                            dst3 = xsrc[:].rearrange("p (c d) -> p c d",
                                                     c=BLK_PAIRS)
                            nc.gpsimd.dma_start(dst3, src)
                        else:
                            xin = xin_pool.tile([128, BLK_PAIRS * D], F32)
                            dst3 = xin[:].rearrange("p (c d) -> p c d",
                                                    c=BLK_PAIRS)
                            nc.gpsimd.dma_start(dst3, src)
                            # rounding cast fp32 -> mm_dt (verifier requires
                            # fp32r matmul inputs to be pre-rounded)
                            xsrc = xbf_pool.tile([128, BLK_PAIRS * D], mm_dt)
                            nc.vector.tensor_copy(xsrc[:], xin[:])

                        for grp in range(BLK_PAIRS // 4):
                            pst = pst_pool.tile([128, 512], mm_dt)
                            xt = xt_pool.tile([128, 512], mm_dt)
                            for k in range(4):
                                c = grp * 4 + k
                                nc.tensor.transpose(
                                    pst[:, k * 128:(k + 1) * 128],
                                    xsrc[:, c * D:(c + 1) * D].bitcast(mm_dt),
                                    ident_mm[:].bitcast(mm_dt),
                                )
                            # PSUM -> SBUF copy of 4 transposed pair-chunks
                            nc.vector.tensor_copy(xt[:], pst[:])

                            psz = psz_pool.tile([128, 1024], F32)
                            for k in range(4):
                                lhsT = xt[:, k * 128:(k + 1) * 128]
                                g2 = (k // 2) * 256
                                rhs = xt[:, g2:g2 + 256]
                                off = k * 256 - (k % 2) * 128
                                nc.tensor.matmul(
                                    psz[:, off:off + 256], lhsT, rhs,
                                    start=True, stop=True,
                                )
                            # extract diag blocks: A rows (partitions 0:64) at
                            # k*256+[0:64]; B rows (64:128) at k*256+[64:128]
                            psz4 = psz[:].rearrange("p (k v) -> p k v", k=4)
                            qq0 = (blk * BLK_PAIRS + grp * 4) * N
                            dst = zbig[:, qq0:qq0 + 256]
                            dstA = dst[0:64].rearrange("p (k v) -> p k v", k=4)
                            dstB = dst[64:128].rearrange("p (k v) -> p k v", k=4)
                            nc.scalar.copy(dstA, psz4[0:64, :, 0:64])
                            nc.scalar.copy(dstB, psz4[64:128, :, 64:128])

                    # packed tril row-DMAs for the whole obatch
                    if skip_out:
                        # timing-only variant: bulk dump, wrong layout
                        flat = zbig[:, 0:OUT_COLS * 2]
                        dst = out[obi * OB:obi * OB + 256]
                        dstv = dst.rearrange("(p r) v -> p (r v)", p=128)
                        nc.sync.dma_start(dstv, flat)
                        continue
                    # zbig[64*a + i, qq*64 + j]; out rows s = ob*OB + 2*qq + a
                    outv = out[obi * OB:(obi + 1) * OB]
                    outv = outv.rearrange("(q a) v -> a q v", a=2)
                    for i in range(1, N):
                        t0 = _tri(i)
                        for a in range(2):
                            srcz = zbig[64 * a + i: 64 * a + i + 1]
                            srcz = srcz.rearrange("p (q j) -> p q j", j=N)
                            # alternate HWDGE rings (SP and ACT) so the
                            # per-dma_start ring overhead halves
                            eng = nc.sync if (i + a) % 2 == 0 else nc.scalar
                            eng.dma_start(
                                outv[a, :, t0:t0 + i].unsqueeze(0),
                                srcz[:, :, 0:i],
                            )

            if repeats == 1:
                body()
            else:
                with tc.For_i(0, repeats, 1) as _i:
                    body(_i)

    nc.compile()
    return nc


_CACHED = {"nc": None, "cfg": None}

# (mode, ob, dma_cast) in preference order; later entries are fallbacks in
# case a config fails compile/verification in the target environment.
_CONFIGS = [
    ("f32r", 512, True),
    ("f32r", 512, False),
    ("bf16", 512, True),
]


def kernel(inputs: np.ndarray) -> np.ndarray:
    """Full-input entry point: inputs [8192, 64, 128] fp32 -> [8192, 2016] fp32."""
    inputs = np.ascontiguousarray(np.asarray(inputs, dtype=np.float32))
    assert inputs.shape == (B_FULL, N, D), inputs.shape
    ident = np.eye(128, dtype=np.float32)
    in_maps = [
        {"x": inputs[c * B_CORE:(c + 1) * B_CORE], "ident": ident}
        for c in range(N_CORES)
    ]
    if _CACHED["nc"] is not None:
        res = bass_utils.run_bass_kernel_spmd(
            _CACHED["nc"], in_maps, core_ids=list(range(N_CORES)))
        return np.concatenate([r["out"] for r in res.results], axis=0)
    last_err = None
    for mode, ob, dc in _CONFIGS:
        try:
            nc = build_nc(mode=mode, ob=ob, dma_cast=dc)
            res = bass_utils.run_bass_kernel_spmd(
                nc, in_maps, core_ids=list(range(N_CORES)))
            _CACHED["nc"] = nc
            _CACHED["cfg"] = (mode, ob, dc)
            return np.concatenate([r["out"] for r in res.results], axis=0)
        except Exception as e:  # compile/verifier failure -> next config
            last_err = e
    raise last_err
